# revision 1
# baseline (speedup 1.0000x reference)
"""GNN message passing (PyG GraphConv, mean aggr) on 8 Trainium2 cores.

Strategy (graph/data parallel, per sharding hint):
 - Nodes sharded contiguously: core c owns nodes [c*6250, (c+1)*6250).
 - Host pre-sorts edges by (core(dst), src_half, window(dst), src) and pads
   each (window, src_half) bucket to a multiple of 128 edges, uniformly
   across cores (single SPMD program).
 - Per layer: dma_gather streams X[src] rows (random 512B HBM reads) into
   SBUF tiles of 128 edges; a one-hot matrix S (built on-device via
   iota==dst_rel, scaled by 1/deg(dst)) turns the segmented mean-reduction
   into PE matmuls accumulating aggT = sum_e X[src_e] outer onehot(dst_e)
   in PSUM, [feat x node] layout.
 - out = agg @ W_rel + b + X @ W_root via two more PE matmuls (aggT / XT as
   stationary), ELU composed as max(x,0) + min(exp(x),1) - 1.
 - X_new shards are AllGather'd between layers (full X needed for gathers).
"""

import numpy as np

N, E, D, L, C = 50000, 600000, 128, 4, 8
NSH = N // C                # 6250 nodes per core
W = 128                     # node window (one-hot width / psum partition)
NW = (NSH + W - 1) // W     # 49 windows per core
HALF = 25000                # gather table split (int16 index limit)
TCH = 8                     # gather chunk size in 128-edge tiles (>8 hangs SWDGE)

_CACHE = {}


def _preprocess(edge_index):
    """Returns (layout, per-core metadata arrays)."""
    src = edge_index[0].astype(np.int64)
    dst = edge_index[1].astype(np.int64)
    deg = np.bincount(dst, minlength=N)
    inv = (1.0 / np.maximum(deg, 1)).astype(np.float32)

    core = dst // NSH
    win = (dst % NSH) // W
    half = (src >= HALF).astype(np.int64)

    # group id per (core, half, window); sort edges by group then src for
    # HBM row locality within each gather run
    g = (core * 2 + half) * NW + win
    order = np.lexsort((src, g))
    gs = g[order]
    NG = C * 2 * NW
    cnt = np.bincount(gs, minlength=NG).reshape(C, 2, NW)

    # uniform tiles per (half, window) across cores
    T_hw = (cnt.max(axis=0) + W - 1) // W          # [2, NW]
    tiles_A, tiles_B = int(T_hw[0].sum()), int(T_hw[1].sum())
    TOT = tiles_A + tiles_B
    # tile base (in tiles) of each (half, window) bucket in the stream
    base = np.zeros((2, NW), np.int64)
    acc = 0
    for h in range(2):
        for w in range(NW):
            base[h, w] = acc
            acc += T_hw[h, w]

    # per-edge destination slot in the padded stream
    group_start = np.zeros(NG + 1, np.int64)
    np.cumsum(np.bincount(gs, minlength=NG), out=group_start[1:])
    rank = np.arange(E) - group_start[gs]
    pos = base[half[order], win[order]] * W + rank  # slot within core stream

    idx_arr = np.zeros((C, TOT * W), np.int16)
    rel_arr = np.full((C, TOT * W), -1.0, np.float32)
    inv_arr = np.zeros((C, TOT * W), np.float32)
    co = core[order]
    idx_arr[co, pos] = (src[order] - half[order] * HALF).astype(np.int16)
    rel_arr[co, pos] = (dst[order] - co * NSH - win[order] * W).astype(np.float32)
    inv_arr[co, pos] = inv[dst[order]]

    # SBUF layouts
    idx_sb = np.tile(
        idx_arr.reshape(C, -1, 16).transpose(0, 2, 1), (1, 8, 1)
    ).copy()                                        # [C, 128, TOT*8]
    rel_sb = rel_arr.reshape(C, TOT, W).transpose(0, 2, 1).copy()  # [C,128,TOT]
    inv_sb = inv_arr.reshape(C, TOT, W).transpose(0, 2, 1).copy()

    layout = dict(T_hw=T_hw, tiles_A=tiles_A, tiles_B=tiles_B, TOT=TOT)
    return layout, idx_sb, rel_sb, inv_sb


def _build(layout):
    import sys
    if "/opt/trn_rl_repo" not in sys.path:
        sys.path.insert(0, "/opt/trn_rl_repo")
    from concourse import bacc, tile, mybir

    f32 = mybir.dt.float32
    T_hw, TOT = layout["T_hw"], layout["TOT"]
    tiles_A = layout["tiles_A"]

    nc = bacc.Bacc("TRN2", target_bir_lowering=False, debug=False,
                   num_devices=C)
    t_x0 = nc.dram_tensor("x0", [N, D], f32, kind="ExternalInput")
    t_x0t = nc.dram_tensor("x0t", [D, NSH], f32, kind="ExternalInput")
    t_idx = nc.dram_tensor("idx", [128, TOT * 8], mybir.dt.int16,
                           kind="ExternalInput")
    t_rel = nc.dram_tensor("rel", [128, TOT], f32, kind="ExternalInput")
    t_inv = nc.dram_tensor("inv", [128, TOT], f32, kind="ExternalInput")
    t_wrel = nc.dram_tensor("wrel", [L, D, D], f32, kind="ExternalInput")
    t_wroot = nc.dram_tensor("wroot", [L, D, D], f32, kind="ExternalInput")
    t_brel = nc.dram_tensor("brel", [1, L * D], f32, kind="ExternalInput")
    t_iota = nc.dram_tensor("iota", [128, 128], f32, kind="ExternalInput")
    t_ident = nc.dram_tensor("ident", [128, 128], f32, kind="ExternalInput")
    t_out = nc.dram_tensor("xout", [NSH, D], f32, kind="ExternalOutput")

    # per-stream-tile (window, is_first_in_bucket, is_last_in_bucket)
    tinfo = []
    for h in range(2):
        for w in range(NW):
            for k in range(T_hw[h, w]):
                tinfo.append((w, k == 0, k == T_hw[h, w] - 1))

    with tile.TileContext(nc) as tc:
        with tc.tile_pool(name="const", bufs=1) as cp, \
             tc.tile_pool(name="xt", bufs=2) as xtp, \
             tc.tile_pool(name="agga", bufs=2) as aap, \
             tc.tile_pool(name="gbuf", bufs=2) as gp, \
             tc.tile_pool(name="small", bufs=4) as sp, \
             tc.tile_pool(name="pagg", bufs=4, space="PSUM") as pagg, \
             tc.tile_pool(name="pout", bufs=2, space="PSUM") as pout, \
             tc.tile_pool(name="pxt", bufs=2, space="PSUM") as pxt, \
             tc.tile_pool(name="dram", bufs=2, space="DRAM") as dp:

            idx_sb = cp.tile([128, TOT * 8], mybir.dt.int16)
            nc.sync.dma_start(out=idx_sb[:], in_=t_idx[:])
            rel_sb = cp.tile([128, TOT], f32)
            nc.sync.dma_start(out=rel_sb[:], in_=t_rel[:])
            inv_sb = cp.tile([128, TOT], f32)
            nc.sync.dma_start(out=inv_sb[:], in_=t_inv[:])
            iota_sb = cp.tile([128, 128], f32)
            nc.sync.dma_start(out=iota_sb[:], in_=t_iota[:])
            ident_sb = cp.tile([128, 128], f32)
            nc.sync.dma_start(out=ident_sb[:], in_=t_ident[:])
            wrel_sb = cp.tile([128, L, D], f32)
            nc.sync.dma_start(out=wrel_sb[:],
                              in_=t_wrel[:].rearrange("l p j -> p l j"))
            wroot_sb = cp.tile([128, L, D], f32)
            nc.sync.dma_start(out=wroot_sb[:],
                              in_=t_wroot[:].rearrange("l p j -> p l j"))
            brel_sb = cp.tile([1, L * D], f32)
            nc.sync.dma_start(out=brel_sb[:], in_=t_brel[:])
            ones_sb = cp.tile([1, 128], f32)
            nc.vector.memset(ones_sb[:], 1.0)

            xt_cur = xtp.tile([D, NSH], f32, tag="xt")
            nc.sync.dma_start(out=xt_cur[:], in_=t_x0t[:])

            x_src = t_x0  # gather source for layer 0
            for l in range(L):
                agga = aap.tile([D, NW * W], f32, tag="agga")
                xt_next = xtp.tile([D, NSH], f32, tag="xt", name="xt_next") if l < L - 1 else None
                xnew_dram = dp.tile([NSH, D], f32, tag="xnew", name="xnew_dram") if l < L - 1 else None

                # chunked gather + aggregation matmuls
                chunk_bounds = list(range(0, tiles_A, TCH)) + [tiles_A] + \
                    list(range(tiles_A + TCH, TOT, TCH)) + [TOT]
                chunk_bounds = sorted(set(chunk_bounds))
                psum_w = None
                for c0, c1 in zip(chunk_bounds[:-1], chunk_bounds[1:]):
                    ct = c1 - c0
                    in_ap = x_src[0:HALF, :] if c0 < tiles_A \
                        else x_src[HALF:N, :]
                    gbuf = gp.tile([128, TCH, D], f32, tag="g")
                    nc.gpsimd.dma_gather(
                        gbuf[:, 0:ct, :], in_ap, idx_sb[:, 8 * c0:8 * c1],
                        ct * 128, ct * 128, D,
                    )
                    for t in range(c0, c1):
                        w, first, last = tinfo[t]
                        s_t = sp.tile([128, 128], f32, tag="s")
                        nc.vector.tensor_scalar(
                            s_t[:], iota_sb[:],
                            rel_sb[:, t:t + 1], inv_sb[:, t:t + 1],
                            mybir.AluOpType.is_equal, mybir.AluOpType.mult,
                        )
                        if first:
                            psum_w = pagg.tile([128, 128], f32, tag="pa")
                        nc.tensor.matmul(
                            psum_w[:], gbuf[:, t - c0, :], s_t[:],
                            start=first, stop=last,
                        )
                        if not last:
                            continue
                        ws = slice(w * W, w * W + min(W, NSH - w * W))
                        wn = ws.stop - ws.start
                        if t < tiles_A:  # phase A: stash partial agg
                            nc.vector.tensor_copy(
                                agga[:, w * W:w * W + 128], psum_w[:])
                            continue
                        # phase B done for window w: finish the node block
                        aggt = sp.tile([128, 128], f32, tag="aggt")
                        nc.vector.tensor_tensor(
                            out=aggt[:], in0=psum_w[:],
                            in1=agga[:, w * W:w * W + 128],
                            op=mybir.AluOpType.add)
                        op = pout.tile([128, 128], f32, tag="po")
                        nc.tensor.matmul(op[0:wn, :], aggt[:, 0:wn],
                                         wrel_sb[:, l, :], start=True,
                                         stop=False)
                        nc.tensor.matmul(op[0:wn, :], xt_cur[:, ws],
                                         wroot_sb[:, l, :], start=False,
                                         stop=False)
                        nc.tensor.matmul(op[0:wn, :], ones_sb[0:1, 0:wn],
                                         brel_sb[0:1, l * D:(l + 1) * D], start=False,
                                         stop=True)
                        # ELU = max(x,0) + min(exp(x),1) - 1
                        e_t = sp.tile([128, 128], f32, tag="e")
                        nc.scalar.activation(
                            e_t[0:wn, :], op[0:wn, :],
                            mybir.ActivationFunctionType.Exp)
                        xr_t = sp.tile([128, 128], f32, tag="xr")
                        nc.scalar.activation(
                            xr_t[0:wn, :], op[0:wn, :],
                            mybir.ActivationFunctionType.Relu)
                        xnew = sp.tile([128, 128], f32, tag="xn")
                        nc.vector.tensor_scalar(
                            xnew[0:wn, :], e_t[0:wn, :], 1.0, 1.0,
                            mybir.AluOpType.min, mybir.AluOpType.subtract)
                        nc.vector.tensor_tensor(
                            out=xnew[0:wn, :], in0=xnew[0:wn, :],
                            in1=xr_t[0:wn, :], op=mybir.AluOpType.add)
                        dst_rows = t_out if l == L - 1 else xnew_dram
                        nc.sync.dma_start(out=dst_rows[ws, :],
                                          in_=xnew[0:wn, :])
                        if l < L - 1:
                            pt = pxt.tile([128, 128], f32, tag="pt")
                            nc.tensor.transpose(pt[:, 0:wn], xnew[0:wn, :],
                                                ident_sb[0:wn, 0:wn])
                            nc.vector.tensor_copy(xt_next[:, ws],
                                                  pt[:, 0:wn])

                if l < L - 1:
                    x_ag = dp.tile([N, D], f32, tag="xag")
                    nc.gpsimd.collective_compute(
                        "AllGather", mybir.AluOpType.bypass,
                        replica_groups=[list(range(C))],
                        ins=[xnew_dram[:].opt()], outs=[x_ag[:].opt()],
                    )
                    x_src = x_ag
                    xt_cur = xt_next

    nc.compile()
    return nc


def kernel(node_embedding, edge_index, Ws_rel, bs_rel, Ws_root):
    import sys
    if "/opt/trn_rl_repo" not in sys.path:
        sys.path.insert(0, "/opt/trn_rl_repo")
    from concourse.bass_utils import run_bass_kernel_spmd

    key = edge_index.tobytes()[:64] + str(edge_index.sum()).encode()
    if key not in _CACHE:
        layout, idx_sb, rel_sb, inv_sb = _preprocess(edge_index)
        nc = _build(layout)
        _CACHE[key] = (nc, idx_sb, rel_sb, inv_sb)
    nc, idx_sb, rel_sb, inv_sb = _CACHE[key]

    x0 = np.ascontiguousarray(node_embedding.astype(np.float32))
    iota = np.broadcast_to(np.arange(128, dtype=np.float32), (128, 128)).copy()
    ident = np.eye(128, dtype=np.float32)
    in_maps = []
    for c in range(C):
        in_maps.append({
            "x0": x0,
            "x0t": np.ascontiguousarray(x0[c * NSH:(c + 1) * NSH].T),
            "idx": idx_sb[c], "rel": rel_sb[c], "inv": inv_sb[c],
            "wrel": np.ascontiguousarray(Ws_rel.astype(np.float32)),
            "wroot": np.ascontiguousarray(Ws_root.astype(np.float32)),
            "brel": np.ascontiguousarray(bs_rel.astype(np.float32).reshape(1, -1)),
            "iota": iota, "ident": ident,
        })
    res = run_bass_kernel_spmd(nc, in_maps, list(range(C)))
    return np.concatenate([res.results[c]["xout"] for c in range(C)], axis=0)



# revision 9
# speedup vs baseline: 1.5868x; 1.5868x over previous
"""GNN message passing (PyG GraphConv, mean aggr) on 8 Trainium2 cores.

Strategy (graph/data parallel):
 - Nodes sharded contiguously: core c owns dst nodes [c*6250, (c+1)*6250).
 - Host sorts edges by (core(dst), src_half, window(dst), src), pads each
   (half, window) bucket to a multiple of 128 edges uniformly across cores
   (single SPMD program). All node data flows in fp16.
 - Per layer: dma_gather streams X[src] rows (256B HBM reads) into SBUF
   tiles of 128 edges; one-hot routing matrices S (iota==dst_rel scaled by
   1/deg) turn the segmented mean into PE matmuls accumulating
   aggT = sum_e X[src_e] outer onehot(dst_e) in PSUM, [feat x node] layout.
   S tiles are layer-invariant: built ONCE on DVE in a pre-phase, parked in
   DRAM, and streamed back per layer — keeps the Vector engine idle during
   layers, where it would otherwise be starved by SWDGE descriptor
   generation (GpSimd shares a SBUF port with DVE).
 - Output computed transposed: outT[fo, node] = Wrel^T aggT + Wroot^T XT
   + b, per 512-node block (one PSUM bank). ELU = Relu(x) - Relu(1-Exp(x))
   using ACT passes and +I/-I PE matmuls (no DVE). outT is directly the
   next layer's XT; row-major X for the next gather is rebuilt with PE
   transposes and written to DRAM, then AllGather'd (fp16, Shared space).
"""

import numpy as np

N, E, D, L, C = 50000, 600000, 128, 4, 8
NSH = N // C                # 6250 nodes per core
W = 128                     # dst window (one-hot width / psum tile)
NW = (NSH + W - 1) // W     # 49 windows per core
HALF = 25000                # gather table split (int16 index limit)
TCH = 8                     # gather chunk size in 128-edge tiles
NBLK = (NSH + 511) // 512   # 13 output blocks of <=512 nodes

_CACHE = {}


def _preprocess(edge_index):
    src = edge_index[0].astype(np.int64)
    dst = edge_index[1].astype(np.int64)
    deg = np.bincount(dst, minlength=N)
    inv = (1.0 / np.maximum(deg, 1)).astype(np.float32)

    core = dst // NSH
    win = (dst % NSH) // W
    half = (src >= HALF).astype(np.int64)

    g = (core * 2 + half) * NW + win
    order = np.lexsort((src, g))
    gs = g[order]
    NG = C * 2 * NW
    cnt = np.bincount(gs, minlength=NG).reshape(C, 2, NW)

    T_hw = (cnt.max(axis=0) + W - 1) // W          # [2, NW] tiles per bucket
    tiles_A, tiles_B = int(T_hw[0].sum()), int(T_hw[1].sum())
    TOT = tiles_A + tiles_B
    base = np.zeros((2, NW), np.int64)
    acc = 0
    for h in range(2):
        for w in range(NW):
            base[h, w] = acc
            acc += T_hw[h, w]

    group_start = np.zeros(NG + 1, np.int64)
    np.cumsum(np.bincount(gs, minlength=NG), out=group_start[1:])
    rank = np.arange(E) - group_start[gs]
    pos = base[half[order], win[order]] * W + rank

    idx_arr = np.zeros((C, TOT * W), np.int16)     # pad slots gather row 0
    rel_arr = np.full((C, TOT * W), -1.0, np.float32)  # pad: no iota match
    wgt_arr = np.zeros((C, TOT * W), np.float32)
    co = core[order]
    idx_arr[co, pos] = (src[order] - half[order] * HALF).astype(np.int16)
    rel_arr[co, pos] = (dst[order] - co * NSH - win[order] * W).astype(np.float32)
    wgt_arr[co, pos] = inv[dst[order]].astype(np.float32)

    idx_sb = np.tile(
        idx_arr.reshape(C, -1, 16).transpose(0, 2, 1), (1, 8, 1)
    ).copy()                                        # [C, 128, TOT*8]
    rel_sb = rel_arr.reshape(C, TOT, W).transpose(0, 2, 1).copy()  # [C,128,TOT]
    wgt_sb = wgt_arr.reshape(C, TOT, W).transpose(0, 2, 1).copy()

    layout = dict(T_hw=T_hw, tiles_A=tiles_A, tiles_B=tiles_B, TOT=TOT)
    return layout, idx_sb, rel_sb, wgt_sb


def _build(layout):
    import sys
    if "/opt/trn_rl_repo" not in sys.path:
        sys.path.insert(0, "/opt/trn_rl_repo")
    from concourse import bacc, tile, mybir

    f32 = mybir.dt.float32
    f16 = mybir.dt.float16
    T_hw, TOT = layout["T_hw"], layout["TOT"]
    tiles_A = layout["tiles_A"]

    nc = bacc.Bacc("TRN2", target_bir_lowering=False, debug=False,
                   num_devices=C)
    t_x0 = nc.dram_tensor("x0h", [N, D], f16, kind="ExternalInput")
    t_x0t = nc.dram_tensor("x0t", [D, NSH], f16, kind="ExternalInput")
    t_idx = nc.dram_tensor("idx", [128, TOT * 8], mybir.dt.int16,
                           kind="ExternalInput")
    t_rel = nc.dram_tensor("rel", [128, TOT], f32, kind="ExternalInput")
    t_wgt = nc.dram_tensor("wgt", [128, TOT], f32, kind="ExternalInput")
    t_wrel = nc.dram_tensor("wrel", [L, D, D], f16, kind="ExternalInput")
    t_wroot = nc.dram_tensor("wroot", [L, D, D], f16, kind="ExternalInput")
    t_brel = nc.dram_tensor("brel", [1, L * D], f16, kind="ExternalInput")
    t_iota = nc.dram_tensor("iota", [128, 128], f16, kind="ExternalInput")
    t_ident = nc.dram_tensor("ident", [128, 128], f16, kind="ExternalInput")
    t_nident = nc.dram_tensor("nident", [128, 128], f16, kind="ExternalInput")
    t_out = nc.dram_tensor("xout", [NSH, D], f32, kind="ExternalOutput")

    # per-stream-tile (window, is_first_in_bucket, is_last_in_bucket)
    tinfo = []
    for h in range(2):
        for w in range(NW):
            for k in range(T_hw[h, w]):
                tinfo.append((w, k == 0, k == T_hw[h, w] - 1))

    # gather / S-stream chunk bounds (phase A chunks, then phase B chunks)
    chunk_bounds = list(range(0, tiles_A, TCH)) + [tiles_A] + \
        list(range(tiles_A + TCH, TOT, TCH)) + [TOT]
    chunk_bounds = sorted(set(chunk_bounds))

    ACT = mybir.ActivationFunctionType

    with tile.TileContext(nc) as tc:
        with tc.tile_pool(name="const", bufs=1) as cp, \
             tc.tile_pool(name="xt", bufs=2) as xtp, \
             tc.tile_pool(name="agg", bufs=2) as aggp, \
             tc.tile_pool(name="gbuf", bufs=3) as gp, \
             tc.tile_pool(name="sbuild", bufs=2) as sbp, \
             tc.tile_pool(name="sstream", bufs=3) as ssp, \
             tc.tile_pool(name="elu", bufs=4) as ep, \
             tc.tile_pool(name="xrow", bufs=3) as xrp, \
             tc.tile_pool(name="pagg", bufs=2, space="PSUM") as pagg, \
             tc.tile_pool(name="pout", bufs=2, space="PSUM") as pout, \
             tc.tile_pool(name="pelu", bufs=2, space="PSUM") as pelu, \
             tc.tile_pool(name="pxt", bufs=2, space="PSUM") as pxt, \
             tc.tile_pool(name="dram", bufs=2, space="DRAM") as dp:

            idx_sb = cp.tile([128, TOT * 8], mybir.dt.int16)
            nc.sync.dma_start(out=idx_sb[:], in_=t_idx[:])
            rel_sb = cp.tile([128, TOT], f32)
            nc.sync.dma_start(out=rel_sb[:], in_=t_rel[:])
            wgt_sb = cp.tile([128, TOT], f32)
            nc.sync.dma_start(out=wgt_sb[:], in_=t_wgt[:])
            iota_sb = cp.tile([128, 128], f16)
            nc.sync.dma_start(out=iota_sb[:], in_=t_iota[:])
            ident_sb = cp.tile([128, 128], f16)
            nc.sync.dma_start(out=ident_sb[:], in_=t_ident[:])
            nident_sb = cp.tile([128, 128], f16)
            nc.sync.dma_start(out=nident_sb[:], in_=t_nident[:])
            wrel_sb = cp.tile([128, L, D], f16)
            nc.sync.dma_start(out=wrel_sb[:],
                              in_=t_wrel[:].rearrange("l p j -> p l j"))
            wroot_sb = cp.tile([128, L, D], f16)
            nc.sync.dma_start(out=wroot_sb[:],
                              in_=t_wroot[:].rearrange("l p j -> p l j"))
            brel_sb = cp.tile([1, L * D], f16)
            nc.sync.dma_start(out=brel_sb[:], in_=t_brel[:])
            ones_sb = cp.tile([1, 512], f16)
            nc.vector.memset(ones_sb[:], 1.0)

            xt_cur = xtp.tile([D, NSH], f16, tag="xt")
            nc.sync.dma_start(out=xt_cur[:], in_=t_x0t[:])

            # ---- pre-phase: build all S tiles once, park in DRAM ----
            s_store = dp.tile([128, TOT * 128], f16, tag="sstore")
            for c0, c1 in zip(chunk_bounds[:-1], chunk_bounds[1:]):
                ct = c1 - c0
                stage = sbp.tile([128, TCH * 128], f16, tag="stage")
                for t in range(c0, c1):
                    j = (t - c0) * 128
                    nc.vector.tensor_scalar(
                        stage[:, j:j + 128], iota_sb[:],
                        rel_sb[:, t:t + 1], wgt_sb[:, t:t + 1],
                        mybir.AluOpType.is_equal, mybir.AluOpType.mult,
                    )
                nc.sync.dma_start(out=s_store[:, c0 * 128:c1 * 128],
                                  in_=stage[:, 0:ct * 128])

            x_src = t_x0
            for l in range(L):
                agga = aggp.tile([D, NW * W], f16, tag="agga")
                aggt = aggp.tile([D, NW * W], f16, tag="aggt", name="aggt")
                xt_next = xtp.tile([D, NSH], f16, tag="xt", name="xt_next")
                xnew_dram = dp.tile([NSH, D], f16, tag="xnew",
                                    name="xnew_dram") if l < L - 1 else None

                nblocks_done = 0
                psum_w = None
                for c0, c1 in zip(chunk_bounds[:-1], chunk_bounds[1:]):
                    ct = c1 - c0
                    in_ap = x_src[0:HALF, :] if c0 < tiles_A \
                        else x_src[HALF:N, :]
                    gbuf = gp.tile([128, TCH, D], f16, tag="g")
                    nc.gpsimd.dma_gather(
                        gbuf[:, 0:ct, :], in_ap, idx_sb[:, 8 * c0:8 * c1],
                        ct * 128, ct * 128, D,
                    )
                    s_sb = ssp.tile([128, TCH * 128], f16, tag="s")
                    nc.sync.dma_start(out=s_sb[:, 0:ct * 128],
                                      in_=s_store[:, c0 * 128:c1 * 128])
                    for t in range(c0, c1):
                        w, first, last = tinfo[t]
                        j = (t - c0) * 128
                        if first:
                            psum_w = pagg.tile([128, 128], f32, tag="pa")
                        phase_a = t < tiles_A
                        nc.tensor.matmul(
                            psum_w[:], gbuf[:, t - c0, :],
                            s_sb[:, j:j + 128],
                            start=first, stop=last and phase_a,
                        )
                        if not last:
                            continue
                        ws = slice(w * W, w * W + min(W, NSH - w * W))
                        wn = ws.stop - ws.start
                        if phase_a:
                            # phase A: park partial aggT (fp16) via ACT
                            nc.scalar.activation(
                                agga[:, w * W:w * W + 128], psum_w[:],
                                ACT.Copy)
                            continue
                        # phase B: add phase-A partial, then close window
                        nc.tensor.matmul(
                            psum_w[:], ident_sb[:],
                            agga[:, w * W:w * W + 128],
                            start=False, stop=True,
                        )
                        nc.scalar.activation(
                            aggt[:, w * W:w * W + 128], psum_w[:], ACT.Copy)

                        # finalize output blocks of 4 windows (512 nodes)
                        while (w + 1) * W >= (nblocks_done + 1) * 512 or \
                              (w == NW - 1 and nblocks_done < NBLK):
                            b = nblocks_done
                            bs = slice(b * 512, min((b + 1) * 512, NSH))
                            bn = bs.stop - bs.start
                            po = pout.tile([128, 512], f32, tag="po")
                            nc.tensor.matmul(
                                po[:, 0:bn], wrel_sb[:, l, :],
                                aggt[:, bs], start=True, stop=False)
                            nc.tensor.matmul(
                                po[:, 0:bn], wroot_sb[:, l, :],
                                xt_cur[:, bs], start=False, stop=False)
                            nc.tensor.matmul(
                                po[:, 0:bn],
                                brel_sb[0:1, l * D:(l + 1) * D],
                                ones_sb[0:1, 0:bn], start=False, stop=True)
                            # ELU = Relu(x) - Relu(1 - Exp(x)), on ACT + PE
                            e_t = ep.tile([128, 512], f16, tag="e")
                            nc.scalar.activation(e_t[:, 0:bn], po[:, 0:bn],
                                                 ACT.Exp)
                            u_t = ep.tile([128, 512], f16, tag="u")
                            nc.scalar.activation(u_t[:, 0:bn], e_t[:, 0:bn],
                                                 ACT.Relu, bias=1.0,
                                                 scale=-1.0)
                            r_t = ep.tile([128, 512], f16, tag="r")
                            nc.scalar.activation(r_t[:, 0:bn], po[:, 0:bn],
                                                 ACT.Relu)
                            p2 = pelu.tile([128, 512], f32, tag="p2")
                            nc.tensor.matmul(p2[:, 0:bn], ident_sb[:],
                                             r_t[:, 0:bn], start=True,
                                             stop=False)
                            nc.tensor.matmul(p2[:, 0:bn], nident_sb[:],
                                             u_t[:, 0:bn], start=False,
                                             stop=True)
                            nc.scalar.activation(xt_next[:, bs], p2[:, 0:bn],
                                                 ACT.Copy)
                            # row-major rebuild per 128-node sub-block
                            for s0 in range(bs.start, bs.stop, 128):
                                sn = min(128, bs.stop - s0)
                                pt = pxt.tile([128, 128], f16, tag="pt")
                                nc.tensor.transpose(
                                    pt[0:sn, :], xt_next[:, s0:s0 + sn],
                                    ident_sb[:])
                                if l < L - 1:
                                    xr = xrp.tile([128, 128], f16, tag="xr")
                                    nc.scalar.activation(xr[0:sn, :],
                                                         pt[0:sn, :],
                                                         ACT.Copy)
                                    nc.sync.dma_start(
                                        out=xnew_dram[s0:s0 + sn, :],
                                        in_=xr[0:sn, :])
                                else:
                                    xo = xrp.tile([128, 128], f32, tag="xo")
                                    nc.scalar.activation(xo[0:sn, :],
                                                         pt[0:sn, :],
                                                         ACT.Copy)
                                    nc.sync.dma_start(
                                        out=t_out[s0:s0 + sn, :],
                                        in_=xo[0:sn, :])
                            nblocks_done += 1
                            if nblocks_done == NBLK:
                                break

                if l < L - 1:
                    x_ag = dp.tile([N, D], f16, tag="xag",
                                   addr_space="Shared")
                    nc.gpsimd.collective_compute(
                        "AllGather", mybir.AluOpType.bypass,
                        replica_groups=[list(range(C))],
                        ins=[xnew_dram[:].opt()], outs=[x_ag[:].opt()],
                    )
                    x_src = x_ag
                    xt_cur = xt_next

    nc.compile()
    return nc


def _make_in_maps(node_embedding, edge_index, Ws_rel, bs_rel, Ws_root,
                  idx_sb, rel_sb, wgt_sb):
    x0 = np.ascontiguousarray(node_embedding.astype(np.float32))
    x0h = x0.astype(np.float16)
    iota = np.broadcast_to(np.arange(128, dtype=np.float16),
                           (128, 128)).copy()
    ident = np.eye(128, dtype=np.float16)
    nident = (-np.eye(128)).astype(np.float16)
    in_maps = []
    for c in range(C):
        in_maps.append({
            "x0h": x0h,
            "x0t": np.ascontiguousarray(x0h[c * NSH:(c + 1) * NSH].T),
            "idx": idx_sb[c], "rel": rel_sb[c], "wgt": wgt_sb[c],
            "wrel": np.ascontiguousarray(Ws_rel.astype(np.float16)),
            "wroot": np.ascontiguousarray(Ws_root.astype(np.float16)),
            "brel": np.ascontiguousarray(
                bs_rel.astype(np.float16).reshape(1, -1)),
            "iota": iota, "ident": ident, "nident": nident,
        })
    return in_maps


def kernel(node_embedding, edge_index, Ws_rel, bs_rel, Ws_root):
    import sys
    if "/opt/trn_rl_repo" not in sys.path:
        sys.path.insert(0, "/opt/trn_rl_repo")
    from concourse.bass_utils import run_bass_kernel_spmd

    key = edge_index.tobytes()[:64] + str(edge_index.sum()).encode()
    if key not in _CACHE:
        layout, idx_sb, rel_sb, wgt_sb = _preprocess(edge_index)
        nc = _build(layout)
        _CACHE[key] = (nc, idx_sb, rel_sb, wgt_sb)
    nc, idx_sb, rel_sb, wgt_sb = _CACHE[key]

    in_maps = _make_in_maps(node_embedding, edge_index, Ws_rel, bs_rel,
                            Ws_root, idx_sb, rel_sb, wgt_sb)
    res = run_bass_kernel_spmd(nc, in_maps, list(range(C)))
    return np.concatenate([res.results[c]["xout"] for c in range(C)], axis=0)
